# revision 7
# baseline (speedup 1.0000x reference)
"""Trainium2 Bass kernel for nn_MultiHeadAttention (B=4, S=2048, H=16, D=64).

Sharding: 8 cores = 4 batches x 2 head-groups (8 heads each). Attention is
fully local per core; the output projection is column-sharded after pairwise
AllGathers (bf16, split 4 ways across head-pairs) of per-head attention
outputs between a batch's two cores.

Math folds (all exact):
- Q and K projections folded into one host-side Q projection:
  energy^T = kt^T (m32^T q + c 1^T) with m32 = wk^T wq / 32, c = wk^T bq / 32
  (the /32 is the softmax 1/sqrt(d_model) scale). K is consumed RAW.
- k bias cancels in softmax (per-column constant).
- V projection folded PAST attention: O = wv (Vnat^T P); 64 ones-columns in
  natural-layout V make PSUM rows 64..127 of R = Vnat^T P the softmax
  denominator, so normalization is a plain reciprocal+mul (no broadcast).
- v bias folds into bo_eff = bo + wo @ tile(bv) host-side.

Pipelining: PE instruction order per k-block is [QK(kb), PV(kb-1)] so the
exp (ScalarE) of block kb overlaps the QK of kb+1 - the kernel runs at the
ScalarE exp roofline (~1.15us per 128x1024 block). The wv-projection of each
query-block is deferred and emitted inside the next block's pipeline.
fp32r for attention math; bf16 only for the output-projection operands.
"""

import numpy as np
import ml_dtypes

import concourse.bass as bass
import concourse.mybir as mybir
import concourse.tile as tile
from concourse import bacc
from concourse.bass_utils import run_bass_kernel_spmd

f32 = mybir.dt.float32
f32r = mybir.dt.float32r
bf16 = mybir.dt.bfloat16

B, S, H, D = 4, 2048, 16, 64
HPC = 8  # heads per core
QB = 1024  # query block (psum: [128, QB] f32 = 2 banks)
NKB = S // 128  # 16 key blocks
EXP = mybir.ActivationFunctionType.Exp
# wo input-dim chunk order matching AllGather arrival (see _prep_core_inputs)
CHUNK_PERM = [0, 4, 1, 5, 2, 6, 3, 7]


def round_fp32r(x: np.ndarray) -> np.ndarray:
    b = np.ascontiguousarray(x.astype(np.float32)).view(np.uint32)
    return ((b + 0x800) & 0xFFFFF000).view(np.float32)


def build(reps=1, use_cc=True):
    nc = bacc.Bacc("TRN2", target_bir_lowering=False, num_devices=8)

    kt = nc.dram_tensor("kt", [HPC, D, S], bf16, kind="ExternalInput")
    qp = nc.dram_tensor("qp", [HPC, D, S], bf16, kind="ExternalInput")
    vna = nc.dram_tensor("vna", [HPC, 128, NKB * 128], bf16, kind="ExternalInput")
    wvt = nc.dram_tensor("wvt", [D, D], f32r, kind="ExternalInput")
    wot = nc.dram_tensor("wot", [HPC, 128, 512], bf16, kind="ExternalInput")
    boe = nc.dram_tensor("boe", [128, 4], f32, kind="ExternalInput")
    out = nc.dram_tensor("out", [512, S], f32, kind="ExternalOutput")

    with tile.TileContext(nc) as tc:
        with tc.tile_pool(name="dram", bufs=1, space="DRAM") as dram:
            for r in range(reps):
                ccin = [
                    dram.tile([128, S], bf16, tag=f"ccin{r}_{j}", name=f"ccin{r}_{j}")
                    for j in range(4)
                ]
                ccout = [
                    dram.tile([256, S], bf16, tag=f"ccout{r}_{j}", name=f"ccout{r}_{j}")
                    for j in range(4)
                ]
                _mha(nc, tc, kt, qp, vna, wvt, wot, boe, out, ccin, ccout, use_cc)
    nc.compile()
    return nc


def _mha(nc, tc, kt, qp, vna, wvt, wot, boe, out, ccin, ccout, use_cc):
    with tc.tile_pool(name="keep", bufs=1) as keep:
        wvt_s = keep.tile([D, D], f32r, tag="wvt")
        nc.default_dma_engine.dma_start(out=wvt_s, in_=wvt[:])
        boe_s = keep.tile([128, 4], f32, tag="boe")
        nc.default_dma_engine.dma_start(out=boe_s, in_=boe[:])
        wo_s = []
        for t in range(HPC):
            w = keep.tile([128, 512], bf16, tag=f"wo{t}", name=f"wo{t}")
            nc.default_dma_engine.dma_start(out=w, in_=wot[t])
            wo_s.append(w)
        _attn(nc, tc, kt, qp, vna, wvt_s, ccin, ccout, use_cc)
        _proj(nc, tc, wo_s, boe_s, out, ccout)


def _attn(nc, tc, kt, qp, vna, wvt_s, ccin, ccout, use_cc):
    with (
        tc.tile_pool(name="hdr", bufs=3) as hdr,
        tc.tile_pool(name="scp", bufs=2, space="PSUM") as scp,
        tc.tile_pool(name="rp", bufs=2, space="PSUM") as rp,
        tc.tile_pool(name="pex", bufs=3) as pex,
        tc.tile_pool(name="nrm", bufs=2) as nrm,
        tc.tile_pool(name="onp", bufs=2) as onp,
    ):
        tiles = {}

        def load_head(p):
            k_t = hdr.tile([D, S], bf16, tag="kt", name=f"kt{p}")
            nc.default_dma_engine.dma_start(out=k_t, in_=kt[p])
            q_t = hdr.tile([D, S], bf16, tag="qp", name=f"qp{p}")
            nc.default_dma_engine.dma_start(out=q_t, in_=qp[p])
            vn_t = hdr.tile([128, NKB, 128], bf16, tag="vn", name=f"vn{p}")
            nc.default_dma_engine.dma_start(
                out=vn_t, in_=vna[p].rearrange("p (n d) -> p n d", n=NKB)
            )
            tiles[p] = (k_t, q_t, vn_t)

        load_head(0)
        load_head(1)
        load_head(2)

        pending = [None]

        def flush():
            if pending[0] is not None:
                pending[0]()
                pending[0] = None

        on_tiles = {}
        for p in range(HPC):
            k_t, q_t, vn_t = tiles.pop(p)
            j = p // 2
            if p % 2 == 0:
                on_tiles[j] = onp.tile([128, S], bf16, tag="On", name=f"On{j}")
            On = on_tiles[j]
            for qb in range(2):
                q0 = qb * QB
                R = rp.tile([128, QB], f32, tag="rp", name=f"R{p}_{qb}")
                pts = {}

                def pv(kb, R=R, pts=pts, vn_t=vn_t):
                    pt = pts.pop(kb)
                    for h in range(QB // 512):
                        nc.tensor.matmul(
                            R[:, h * 512 : (h + 1) * 512],
                            lhsT=vn_t[:, kb, :],
                            rhs=pt[:, h * 512 : (h + 1) * 512],
                            start=(kb == 0),
                            stop=(kb == NKB - 1),
                        )

                for kb in range(NKB):
                    sc = scp.tile([128, QB], f32, tag="sc")
                    for h in range(QB // 512):
                        nc.tensor.matmul(
                            sc[:, h * 512 : (h + 1) * 512],
                            lhsT=k_t[:, kb * 128 : (kb + 1) * 128],
                            rhs=q_t[:, q0 + h * 512 : q0 + (h + 1) * 512],
                            start=True,
                            stop=True,
                        )
                    if kb == 2:
                        flush()
                    if kb == 4 and qb == 0 and p + 2 < HPC:
                        load_head(p + 2)
                    pt = pex.tile([128, QB], bf16, tag="pt")
                    nc.scalar.activation(pt[:], sc[:], EXP, scale=1.0)
                    pts[kb] = pt
                    if kb > 0:
                        pv(kb - 1)
                pv(NKB - 1)

                bcs = nrm.tile([D, QB], f32r, tag="bcs")
                with nc.allow_low_precision(reason="fp32r softmax denom"):
                    nc.vector.reciprocal(bcs[:], R[D : 2 * D, :])
                rsb = nrm.tile([D, QB], f32r, tag="rsb")
                nc.vector.tensor_mul(rsb[:], R[0:D, :], bcs[:])

                def mk(p=p, qb=qb, q0=q0, rsb=rsb, On=On, j=j):
                    def go():
                        ops = rp.tile([128, QB], f32, tag="rp", name=f"ops{p}_{qb}")
                        for h in range(QB // 512):
                            nc.tensor.matmul(
                                ops[0:D, h * 512 : (h + 1) * 512],
                                lhsT=wvt_s[:],
                                rhs=rsb[:, h * 512 : (h + 1) * 512],
                                start=True,
                                stop=True,
                            )
                        row = (p % 2) * D
                        with nc.allow_low_precision(reason="bf16 cc exchange"):
                            nc.vector.tensor_copy(
                                On[row : row + D, q0 : q0 + QB], ops[0:D, :]
                            )
                        if p % 2 == 1 and qb == 1:
                            nc.gpsimd.dma_start(out=ccin[j], in_=On[:])
                            if use_cc:
                                nc.gpsimd.collective_compute(
                                    "AllGather",
                                    mybir.AluOpType.bypass,
                                    replica_groups=[[0, 1], [2, 3], [4, 5], [6, 7]],
                                    ins=[ccin[j].opt()],
                                    outs=[ccout[j].opt()],
                                )

                    return go

                pending[0] = mk()
        flush()


def _proj(nc, tc, wo_s, boe_s, out, ccout):
    with (
        tc.tile_pool(name="pcc", bufs=1) as pcc,
        tc.tile_pool(name="fin", bufs=2) as finp,
        tc.tile_pool(name="fps", bufs=2, space="PSUM") as fpsp,
    ):
        ch = []
        for j in range(4):
            for half in range(2):
                t = pcc.tile([128, S], bf16, tag=f"ch{2 * j + half}")
                nc.default_dma_engine.dma_start(
                    out=t, in_=ccout[j][half * 128 : (half + 1) * 128, :]
                )
                ch.append(t)
        for ob in range(4):
            for qc in range(4):
                fp_ = fpsp.tile([128, 512], f32, tag="fp")
                for t in range(HPC):
                    nc.tensor.matmul(
                        fp_[:],
                        lhsT=wo_s[t][:, ob * 128 : (ob + 1) * 128],
                        rhs=ch[t][:, qc * 512 : (qc + 1) * 512],
                        start=(t == 0),
                        stop=(t == HPC - 1),
                    )
                fo = finp.tile([128, 512], f32, tag="fo")
                nc.vector.tensor_scalar_add(fo[:], fp_[:], boe_s[:, ob : ob + 1])
                nc.default_dma_engine.dma_start(
                    out=out[ob * 128 : (ob + 1) * 128, qc * 512 : (qc + 1) * 512],
                    in_=fo[:],
                )


_NC_CACHE = {}


def _get_nc(reps=1, use_cc=True):
    key = (reps, use_cc)
    if key not in _NC_CACHE:
        _NC_CACHE[key] = build(reps, use_cc)
    return _NC_CACHE[key]


def _prep_core_inputs(values, keys, query, wq, bq, wk, bk, wv, bv, wo, bo):
    """Build the 8 per-core input maps (host-side shard + layout prep)."""
    del bk  # cancels in softmax (per-column constant)
    m32 = (wq.T.astype(np.float64) @ wk.astype(np.float64)) / 32.0
    cvec = (wk.T.astype(np.float64) @ bq.astype(np.float64)) / 32.0
    wvt = round_fp32r(wv.T)

    bv_full = np.tile(bv, H)
    bo_eff = (
        bo.astype(np.float64) + wo.astype(np.float64) @ bv_full.astype(np.float64)
    ).astype(np.float32)
    woT = wo.T  # [in 1024, out 1024]

    # host-side Q projection: qp = q @ m32 + c  (folds wq, wk, bq, 1/32 scale)
    qall = query.reshape(B, S, H, D).astype(np.float32)
    qp_all = (
        qall.reshape(-1, D) @ m32.astype(np.float32)
    ).reshape(B, S, H, D) + cvec.astype(np.float32)

    in_maps = []
    ones = np.ones((HPC, NKB, 128, D), np.float32)
    for c in range(8):
        b, g = c // 2, c % 2
        heads = slice(g * HPC, (g + 1) * HPC)
        ktc = keys[b].reshape(S, H, D)[:, heads, :].transpose(1, 2, 0)
        qpc = qp_all[b][:, heads, :].transpose(1, 2, 0)
        vn = values[b].reshape(S, H, D)[:, heads, :].transpose(1, 0, 2)
        vn = vn.reshape(HPC, NKB, 128, D)
        vna = np.concatenate([vn, ones], axis=3)  # [h, kb, 128key, 128col]
        vna = vna.transpose(0, 2, 1, 3).reshape(HPC, 128, NKB * 128)
        ocols = slice(g * 512, (g + 1) * 512)
        wot = woT[:, ocols].reshape(HPC, 128, 512)[CHUNK_PERM]
        boe = np.ascontiguousarray(bo_eff[g * 512 : (g + 1) * 512].reshape(4, 128).T)
        in_maps.append(
            dict(
                kt=np.ascontiguousarray(ktc).astype(ml_dtypes.bfloat16),
                qp=np.ascontiguousarray(qpc).astype(ml_dtypes.bfloat16),
                vna=np.ascontiguousarray(vna).astype(ml_dtypes.bfloat16),
                wvt=wvt,
                wot=np.ascontiguousarray(wot).astype(ml_dtypes.bfloat16),
                boe=boe,
            )
        )
    return in_maps


def kernel(values, keys, query, wq, bq, wk, bk, wv, bv, wo, bo):
    values = np.asarray(values, np.float32)
    keys = np.asarray(keys, np.float32)
    query = np.asarray(query, np.float32)
    in_maps = _prep_core_inputs(
        values, keys, query,
        np.asarray(wq, np.float32), np.asarray(bq, np.float32),
        np.asarray(wk, np.float32), np.asarray(bk, np.float32),
        np.asarray(wv, np.float32), np.asarray(bv, np.float32),
        np.asarray(wo, np.float32), np.asarray(bo, np.float32),
    )
    nc = _get_nc()
    res = run_bass_kernel_spmd(nc, in_maps, list(range(8)))
    out = np.empty((B, S, 1024), np.float32)
    for c in range(8):
        b, g = c // 2, c % 2
        out[b, :, g * 512 : (g + 1) * 512] = res.results[c]["out"].T
    return out


# revision 9
# speedup vs baseline: 1.3598x; 1.3598x over previous
"""Trainium2 Bass kernel for nn_MultiHeadAttention (B=4, S=2048, H=16, D=64).

Sharding: 8 cores = 4 batches x 2 head-groups (8 heads each). Attention is
fully local per core; the output projection is column-sharded after pairwise
AllGathers (bf16, split 4 ways across head-pairs) of per-head attention
outputs between a batch's two cores.

Math folds (all exact):
- Q and K projections folded into one host-side Q projection:
  energy^T = kt^T (m32^T q + c 1^T) with m32 = wk^T wq / 32, c = wk^T bq / 32
  (the /32 is the softmax 1/sqrt(d_model) scale). K is consumed RAW.
- k bias cancels in softmax (per-column constant).
- V projection folded PAST attention: O = wv (Vnat^T P); 64 ones-columns in
  natural-layout V make PSUM rows 64..127 of R = Vnat^T P the softmax
  denominator, so normalization is a plain reciprocal+mul (no broadcast).
- v bias folds into bo_eff = bo + wo @ tile(bv) host-side.

Pipelining: PE instruction order per k-block is [QK(kb), PV(kb-1)] so the
exp (ScalarE) of block kb overlaps the QK of kb+1 - the kernel runs at the
ScalarE exp roofline (~1.15us per 128x1024 block). The wv-projection of each
query-block is deferred and emitted inside the next block's pipeline.
fp32r for attention math; bf16 only for the output-projection operands.
"""

import numpy as np
import ml_dtypes

import concourse.bass as bass
import concourse.mybir as mybir
import concourse.tile as tile
from concourse import bacc
from concourse.bass_utils import run_bass_kernel_spmd

f32 = mybir.dt.float32
f32r = mybir.dt.float32r
bf16 = mybir.dt.bfloat16

B, S, H, D = 4, 2048, 16, 64
HPC = 8  # heads per core
QB = 1024  # query block (psum: [128, QB] f32 = 2 banks)
NKB = S // 128  # 16 key blocks
EXP = mybir.ActivationFunctionType.Exp
# wo input-dim chunk order matching AllGather arrival (see _prep_core_inputs)
CHUNK_PERM = [0, 4, 1, 5, 2, 6, 3, 7]


def round_fp32r(x: np.ndarray) -> np.ndarray:
    b = np.ascontiguousarray(x.astype(np.float32)).view(np.uint32)
    return ((b + 0x800) & 0xFFFFF000).view(np.float32)


def build(reps=1, use_cc=True):
    nc = bacc.Bacc("TRN2", target_bir_lowering=False, num_devices=8)

    kt = nc.dram_tensor("kt", [HPC, D, S], bf16, kind="ExternalInput")
    qp = nc.dram_tensor("qp", [HPC, D, S], bf16, kind="ExternalInput")
    vna = nc.dram_tensor("vna", [HPC, 128, NKB * 128], bf16, kind="ExternalInput")
    wvt = nc.dram_tensor("wvt", [D, D], f32r, kind="ExternalInput")
    wot = nc.dram_tensor("wot", [HPC, 128, 512], bf16, kind="ExternalInput")
    boe = nc.dram_tensor("boe", [128, 4], f32, kind="ExternalInput")
    out = nc.dram_tensor("out", [512, S], f32, kind="ExternalOutput")

    with tile.TileContext(nc) as tc:
        with tc.tile_pool(name="dram", bufs=1, space="DRAM") as dram:
            for r in range(reps):
                ccin = [
                    dram.tile([128, S], bf16, tag=f"ccin{r}_{j}", name=f"ccin{r}_{j}")
                    for j in range(4)
                ]
                ccout = [
                    dram.tile([256, S], bf16, tag=f"ccout{r}_{j}", name=f"ccout{r}_{j}")
                    for j in range(4)
                ]
                _mha(nc, tc, kt, qp, vna, wvt, wot, boe, out, ccin, ccout, use_cc)
    nc.compile()
    return nc


def _mha(nc, tc, kt, qp, vna, wvt, wot, boe, out, ccin, ccout, use_cc):
    with tc.tile_pool(name="keep", bufs=1) as keep:
        wvt_s = keep.tile([D, D], f32r, tag="wvt")
        nc.default_dma_engine.dma_start(out=wvt_s, in_=wvt[:])
        boe_s = keep.tile([128, 4], f32, tag="boe")
        nc.default_dma_engine.dma_start(out=boe_s, in_=boe[:])
        wo_s = []
        for t in range(HPC):
            w = keep.tile([128, 512], bf16, tag=f"wo{t}", name=f"wo{t}")
            nc.default_dma_engine.dma_start(out=w, in_=wot[t])
            wo_s.append(w)
        _attn(nc, tc, kt, qp, vna, wvt_s, ccin, ccout, use_cc)
        _proj(nc, tc, wo_s, boe_s, out, ccout)


def _attn(nc, tc, kt, qp, vna, wvt_s, ccin, ccout, use_cc):
    with (
        tc.tile_pool(name="hdr", bufs=3) as hdr,
        tc.tile_pool(name="scp", bufs=2, space="PSUM") as scp,
        tc.tile_pool(name="rp", bufs=2, space="PSUM") as rp,
        tc.tile_pool(name="pex", bufs=3) as pex,
        tc.tile_pool(name="nrm", bufs=2) as nrm,
        tc.tile_pool(name="onp", bufs=2) as onp,
    ):
        tiles = {}

        def load_head(p):
            k_t = hdr.tile([D, S], bf16, tag="kt", name=f"kt{p}")
            nc.default_dma_engine.dma_start(out=k_t, in_=kt[p])
            q_t = hdr.tile([D, S], bf16, tag="qp", name=f"qp{p}")
            nc.default_dma_engine.dma_start(out=q_t, in_=qp[p])
            vn_t = hdr.tile([128, NKB, 128], bf16, tag="vn", name=f"vn{p}")
            nc.default_dma_engine.dma_start(
                out=vn_t, in_=vna[p].rearrange("p (n d) -> p n d", n=NKB)
            )
            tiles[p] = (k_t, q_t, vn_t)

        load_head(0)
        load_head(1)
        load_head(2)

        pending = [None]

        def flush():
            if pending[0] is not None:
                pending[0]()
                pending[0] = None

        on_tiles = {}
        for p in range(HPC):
            k_t, q_t, vn_t = tiles.pop(p)
            j = p // 2
            if p % 2 == 0:
                on_tiles[j] = onp.tile([128, S], bf16, tag="On", name=f"On{j}")
            On = on_tiles[j]
            for qb in range(2):
                q0 = qb * QB
                R = rp.tile([128, QB], f32, tag="rp", name=f"R{p}_{qb}")
                pts = {}

                def pv(kb, R=R, pts=pts, vn_t=vn_t):
                    pt = pts.pop(kb)
                    for h in range(QB // 512):
                        nc.tensor.matmul(
                            R[:, h * 512 : (h + 1) * 512],
                            lhsT=vn_t[:, kb, :],
                            rhs=pt[:, h * 512 : (h + 1) * 512],
                            start=(kb == 0),
                            stop=(kb == NKB - 1),
                        )

                for kb in range(NKB):
                    sc = scp.tile([128, QB], f32, tag="sc")
                    for h in range(QB // 512):
                        nc.tensor.matmul(
                            sc[:, h * 512 : (h + 1) * 512],
                            lhsT=k_t[:, kb * 128 : (kb + 1) * 128],
                            rhs=q_t[:, q0 + h * 512 : q0 + (h + 1) * 512],
                            start=True,
                            stop=True,
                        )
                    if kb == 2:
                        flush()
                    if kb == 4 and qb == 0 and p + 2 < HPC:
                        load_head(p + 2)
                    pt = pex.tile([128, QB], bf16, tag="pt")
                    nc.scalar.activation(pt[:], sc[:], EXP, scale=1.0)
                    pts[kb] = pt
                    if kb > 0:
                        pv(kb - 1)
                pv(NKB - 1)

                den = nrm.tile([D, QB], f32, tag="den")
                nc.vector.tensor_copy(den[:], R[D : 2 * D, :])
                bcs = nrm.tile([D, QB], f32, tag="bcs")
                nc.vector.reciprocal_approx_fast(bcs[:], den[:])
                rsb = nrm.tile([D, QB], f32r, tag="rsb")
                nc.vector.tensor_mul(rsb[:], R[0:D, :], bcs[:])

                def mk(p=p, qb=qb, q0=q0, rsb=rsb, On=On, j=j):
                    def go():
                        ops = rp.tile([128, QB], f32, tag="rp", name=f"ops{p}_{qb}")
                        for h in range(QB // 512):
                            nc.tensor.matmul(
                                ops[0:D, h * 512 : (h + 1) * 512],
                                lhsT=wvt_s[:],
                                rhs=rsb[:, h * 512 : (h + 1) * 512],
                                start=True,
                                stop=True,
                            )
                        row = (p % 2) * D
                        with nc.allow_low_precision(reason="bf16 cc exchange"):
                            nc.vector.tensor_copy(
                                On[row : row + D, q0 : q0 + QB], ops[0:D, :]
                            )
                        if p % 2 == 1 and qb == 1:
                            nc.gpsimd.dma_start(out=ccin[j], in_=On[:])
                            if use_cc:
                                nc.gpsimd.collective_compute(
                                    "AllGather",
                                    mybir.AluOpType.bypass,
                                    replica_groups=[[0, 1], [2, 3], [4, 5], [6, 7]],
                                    ins=[ccin[j].opt()],
                                    outs=[ccout[j].opt()],
                                )

                    return go

                pending[0] = mk()
        flush()


def _proj(nc, tc, wo_s, boe_s, out, ccout):
    with (
        tc.tile_pool(name="pcc", bufs=1) as pcc,
        tc.tile_pool(name="fin", bufs=2) as finp,
        tc.tile_pool(name="fps", bufs=2, space="PSUM") as fpsp,
    ):
        ch = []
        for j in range(4):
            for half in range(2):
                t = pcc.tile([128, S], bf16, tag=f"ch{2 * j + half}")
                nc.default_dma_engine.dma_start(
                    out=t, in_=ccout[j][half * 128 : (half + 1) * 128, :]
                )
                ch.append(t)
        for ob in range(4):
            for qc in range(4):
                fp_ = fpsp.tile([128, 512], f32, tag="fp")
                for t in range(HPC):
                    nc.tensor.matmul(
                        fp_[:],
                        lhsT=wo_s[t][:, ob * 128 : (ob + 1) * 128],
                        rhs=ch[t][:, qc * 512 : (qc + 1) * 512],
                        start=(t == 0),
                        stop=(t == HPC - 1),
                    )
                fo = finp.tile([128, 512], f32, tag="fo")
                nc.vector.tensor_scalar_add(fo[:], fp_[:], boe_s[:, ob : ob + 1])
                nc.default_dma_engine.dma_start(
                    out=out[ob * 128 : (ob + 1) * 128, qc * 512 : (qc + 1) * 512],
                    in_=fo[:],
                )


_NC_CACHE = {}


def _get_nc(reps=1, use_cc=True):
    key = (reps, use_cc)
    if key not in _NC_CACHE:
        _NC_CACHE[key] = build(reps, use_cc)
    return _NC_CACHE[key]


def _prep_core_inputs(values, keys, query, wq, bq, wk, bk, wv, bv, wo, bo):
    """Build the 8 per-core input maps (host-side shard + layout prep)."""
    del bk  # cancels in softmax (per-column constant)
    m32 = (wq.T.astype(np.float64) @ wk.astype(np.float64)) / 32.0
    cvec = (wk.T.astype(np.float64) @ bq.astype(np.float64)) / 32.0
    wvt = round_fp32r(wv.T)

    bv_full = np.tile(bv, H)
    bo_eff = (
        bo.astype(np.float64) + wo.astype(np.float64) @ bv_full.astype(np.float64)
    ).astype(np.float32)
    woT = wo.T  # [in 1024, out 1024]

    # host-side Q projection: qp = q @ m32 + c  (folds wq, wk, bq, 1/32 scale)
    qall = query.reshape(B, S, H, D).astype(np.float32)
    qp_all = (
        qall.reshape(-1, D) @ m32.astype(np.float32)
    ).reshape(B, S, H, D) + cvec.astype(np.float32)

    in_maps = []
    ones = np.ones((HPC, NKB, 128, D), np.float32)
    for c in range(8):
        b, g = c // 2, c % 2
        heads = slice(g * HPC, (g + 1) * HPC)
        ktc = keys[b].reshape(S, H, D)[:, heads, :].transpose(1, 2, 0)
        qpc = qp_all[b][:, heads, :].transpose(1, 2, 0)
        vn = values[b].reshape(S, H, D)[:, heads, :].transpose(1, 0, 2)
        vn = vn.reshape(HPC, NKB, 128, D)
        vna = np.concatenate([vn, ones], axis=3)  # [h, kb, 128key, 128col]
        vna = vna.transpose(0, 2, 1, 3).reshape(HPC, 128, NKB * 128)
        ocols = slice(g * 512, (g + 1) * 512)
        wot = woT[:, ocols].reshape(HPC, 128, 512)[CHUNK_PERM]
        boe = np.ascontiguousarray(bo_eff[g * 512 : (g + 1) * 512].reshape(4, 128).T)
        in_maps.append(
            dict(
                kt=np.ascontiguousarray(ktc).astype(ml_dtypes.bfloat16),
                qp=np.ascontiguousarray(qpc).astype(ml_dtypes.bfloat16),
                vna=np.ascontiguousarray(vna).astype(ml_dtypes.bfloat16),
                wvt=wvt,
                wot=np.ascontiguousarray(wot).astype(ml_dtypes.bfloat16),
                boe=boe,
            )
        )
    return in_maps


def kernel(values, keys, query, wq, bq, wk, bk, wv, bv, wo, bo):
    values = np.asarray(values, np.float32)
    keys = np.asarray(keys, np.float32)
    query = np.asarray(query, np.float32)
    in_maps = _prep_core_inputs(
        values, keys, query,
        np.asarray(wq, np.float32), np.asarray(bq, np.float32),
        np.asarray(wk, np.float32), np.asarray(bk, np.float32),
        np.asarray(wv, np.float32), np.asarray(bv, np.float32),
        np.asarray(wo, np.float32), np.asarray(bo, np.float32),
    )
    nc = _get_nc()
    res = run_bass_kernel_spmd(nc, in_maps, list(range(8)))
    out = np.empty((B, S, 1024), np.float32)
    for c in range(8):
        b, g = c // 2, c % 2
        out[b, :, g * 512 : (g + 1) * 512] = res.results[c]["out"].T
    return out


# revision 10
# speedup vs baseline: 1.4310x; 1.0524x over previous
"""Trainium2 Bass kernel for nn_MultiHeadAttention (B=4, S=2048, H=16, D=64).

Sharding: 8 cores = 4 batches x 2 head-groups (8 heads each). Attention is
fully local per core; the output projection is column-sharded after pairwise
AllGathers (bf16, split 4 ways across head-pairs) of per-head attention
outputs between a batch's two cores.

Math folds (all exact):
- Q and K projections folded into one host-side Q projection:
  energy^T = kt^T (m32^T q + c 1^T) with m32 = wk^T wq / 32, c = wk^T bq / 32
  (the /32 is the softmax 1/sqrt(d_model) scale). K is consumed RAW.
- k bias cancels in softmax (per-column constant).
- V projection folded PAST attention: O = wv (Vnat^T P); 64 ones-columns in
  natural-layout V make PSUM rows 64..127 of R = Vnat^T P the softmax
  denominator, so normalization is a plain reciprocal+mul (no broadcast).
- v bias folds into bo_eff = bo + wo @ tile(bv) host-side.

Pipelining: PE instruction order per k-block is [QK(kb), PV(kb-1)] so the
exp (ScalarE) of block kb overlaps the QK of kb+1 - the kernel runs at the
ScalarE exp roofline (~1.15us per 128x1024 block). The wv-projection of each
query-block is deferred and emitted inside the next block's pipeline.
fp32r for attention math; bf16 only for the output-projection operands.
"""

import numpy as np
import ml_dtypes

import concourse.bass as bass
import concourse.mybir as mybir
import concourse.tile as tile
from concourse import bacc
from concourse.bass_utils import run_bass_kernel_spmd

f32 = mybir.dt.float32
f32r = mybir.dt.float32r
bf16 = mybir.dt.bfloat16

B, S, H, D = 4, 2048, 16, 64
HPC = 8  # heads per core
QB = 1024  # query block (psum: [128, QB] f32 = 2 banks)
NKB = S // 128  # 16 key blocks
EXP = mybir.ActivationFunctionType.Exp
# wo input-dim chunk order matching AllGather arrival (see _prep_core_inputs)
CHUNK_PERM = [0, 4, 1, 5, 2, 6, 3, 7]


def round_fp32r(x: np.ndarray) -> np.ndarray:
    b = np.ascontiguousarray(x.astype(np.float32)).view(np.uint32)
    return ((b + 0x800) & 0xFFFFF000).view(np.float32)


def build(reps=1, use_cc=True):
    nc = bacc.Bacc("TRN2", target_bir_lowering=False, num_devices=8)

    kt = nc.dram_tensor("kt", [HPC, D, S], bf16, kind="ExternalInput")
    qp = nc.dram_tensor("qp", [HPC, D, S], bf16, kind="ExternalInput")
    vna = nc.dram_tensor("vna", [HPC, 128, NKB * 128], bf16, kind="ExternalInput")
    wvt = nc.dram_tensor("wvt", [D, D], bf16, kind="ExternalInput")
    wot = nc.dram_tensor("wot", [HPC, 128, 512], bf16, kind="ExternalInput")
    boe = nc.dram_tensor("boe", [128, 4], f32, kind="ExternalInput")
    out = nc.dram_tensor("out", [512, S], f32, kind="ExternalOutput")

    with tile.TileContext(nc) as tc:
        with tc.tile_pool(name="dram", bufs=1, space="DRAM") as dram:
            for r in range(reps):
                ccin = [
                    dram.tile([128, S], bf16, tag=f"ccin{r}_{j}", name=f"ccin{r}_{j}")
                    for j in range(4)
                ]
                ccout = [
                    dram.tile([256, S], bf16, tag=f"ccout{r}_{j}", name=f"ccout{r}_{j}")
                    for j in range(4)
                ]
                _mha(nc, tc, kt, qp, vna, wvt, wot, boe, out, ccin, ccout, use_cc)
    nc.compile()
    return nc


def _mha(nc, tc, kt, qp, vna, wvt, wot, boe, out, ccin, ccout, use_cc):
    with tc.tile_pool(name="keep", bufs=1) as keep:
        wvt_s = keep.tile([D, D], bf16, tag="wvt")
        nc.default_dma_engine.dma_start(out=wvt_s, in_=wvt[:])
        boe_s = keep.tile([128, 4], f32, tag="boe")
        nc.default_dma_engine.dma_start(out=boe_s, in_=boe[:])
        wo_s = []
        for t in range(HPC):
            w = keep.tile([128, 512], bf16, tag=f"wo{t}", name=f"wo{t}")
            nc.default_dma_engine.dma_start(out=w, in_=wot[t])
            wo_s.append(w)
        _attn(nc, tc, kt, qp, vna, wvt_s, ccin, ccout, use_cc)
        _proj(nc, tc, wo_s, boe_s, out, ccout)


def _attn(nc, tc, kt, qp, vna, wvt_s, ccin, ccout, use_cc):
    with (
        tc.tile_pool(name="hdr", bufs=3) as hdr,
        tc.tile_pool(name="scp", bufs=2, space="PSUM") as scp,
        tc.tile_pool(name="rp", bufs=2, space="PSUM") as rp,
        tc.tile_pool(name="pex", bufs=3) as pex,
        tc.tile_pool(name="nrm", bufs=2) as nrm,
        tc.tile_pool(name="onp", bufs=2) as onp,
    ):
        tiles = {}

        def load_head(p):
            k_t = hdr.tile([D, S], bf16, tag="kt", name=f"kt{p}")
            nc.default_dma_engine.dma_start(out=k_t, in_=kt[p])
            q_t = hdr.tile([D, S], bf16, tag="qp", name=f"qp{p}")
            nc.default_dma_engine.dma_start(out=q_t, in_=qp[p])
            vn_t = hdr.tile([128, NKB, 128], bf16, tag="vn", name=f"vn{p}")
            nc.default_dma_engine.dma_start(
                out=vn_t, in_=vna[p].rearrange("p (n d) -> p n d", n=NKB)
            )
            tiles[p] = (k_t, q_t, vn_t)

        load_head(0)
        load_head(1)
        load_head(2)

        pending = [None]

        def flush():
            if pending[0] is not None:
                pending[0]()
                pending[0] = None

        on_tiles = {}
        for p in range(HPC):
            k_t, q_t, vn_t = tiles.pop(p)
            j = p // 2
            if p % 2 == 0:
                on_tiles[j] = onp.tile([128, S], bf16, tag="On", name=f"On{j}")
            On = on_tiles[j]
            for qb in range(2):
                q0 = qb * QB
                R = rp.tile([128, QB], f32, tag="rp", name=f"R{p}_{qb}")
                pts = {}

                def pv(kb, R=R, pts=pts, vn_t=vn_t):
                    pt = pts.pop(kb)
                    for h in range(QB // 512):
                        nc.tensor.matmul(
                            R[:, h * 512 : (h + 1) * 512],
                            lhsT=vn_t[:, kb, :],
                            rhs=pt[:, h * 512 : (h + 1) * 512],
                            start=(kb == 0),
                            stop=(kb == NKB - 1),
                        )

                for kb in range(NKB):
                    sc = scp.tile([128, QB], f32, tag="sc")
                    for h in range(QB // 512):
                        nc.tensor.matmul(
                            sc[:, h * 512 : (h + 1) * 512],
                            lhsT=k_t[:, kb * 128 : (kb + 1) * 128],
                            rhs=q_t[:, q0 + h * 512 : q0 + (h + 1) * 512],
                            start=True,
                            stop=True,
                        )
                    if kb == 2:
                        flush()
                    if kb == 4 and qb == 0 and p + 2 < HPC:
                        load_head(p + 2)
                    pt = pex.tile([128, QB], bf16, tag="pt")
                    nc.scalar.activation(pt[:], sc[:], EXP, scale=1.0)
                    pts[kb] = pt
                    if kb > 0:
                        pv(kb - 1)
                pv(NKB - 1)

                den = nrm.tile([D, QB], f32, tag="den")
                nc.vector.tensor_copy(den[:], R[D : 2 * D, :])
                bcs = nrm.tile([D, QB], f32, tag="bcs")
                nc.vector.reciprocal_approx_fast(bcs[:], den[:])
                rsb = nrm.tile([D, QB], bf16, tag="rsb")
                nc.vector.tensor_mul(rsb[:], R[0:D, :], bcs[:])

                def mk(p=p, qb=qb, q0=q0, rsb=rsb, On=On, j=j):
                    def go():
                        ops = rp.tile([128, QB], f32, tag="rp", name=f"ops{p}_{qb}")
                        for h in range(QB // 512):
                            nc.tensor.matmul(
                                ops[0:D, h * 512 : (h + 1) * 512],
                                lhsT=wvt_s[:],
                                rhs=rsb[:, h * 512 : (h + 1) * 512],
                                start=True,
                                stop=True,
                            )
                        row = (p % 2) * D
                        with nc.allow_low_precision(reason="bf16 cc exchange"):
                            nc.vector.tensor_copy(
                                On[row : row + D, q0 : q0 + QB], ops[0:D, :]
                            )
                        if p % 2 == 1 and qb == 1:
                            nc.gpsimd.dma_start(out=ccin[j], in_=On[:])
                            if use_cc:
                                nc.gpsimd.collective_compute(
                                    "AllGather",
                                    mybir.AluOpType.bypass,
                                    replica_groups=[[0, 1], [2, 3], [4, 5], [6, 7]],
                                    ins=[ccin[j].opt()],
                                    outs=[ccout[j].opt()],
                                )

                    return go

                pending[0] = mk()
        flush()


def _proj(nc, tc, wo_s, boe_s, out, ccout):
    with (
        tc.tile_pool(name="pcc", bufs=1) as pcc,
        tc.tile_pool(name="fin", bufs=2) as finp,
        tc.tile_pool(name="fps", bufs=2, space="PSUM") as fpsp,
    ):
        ch = []
        for j in range(4):
            for half in range(2):
                t = pcc.tile([128, S], bf16, tag=f"ch{2 * j + half}")
                nc.default_dma_engine.dma_start(
                    out=t, in_=ccout[j][half * 128 : (half + 1) * 128, :]
                )
                ch.append(t)
        for ob in range(4):
            for qc in range(4):
                fp_ = fpsp.tile([128, 512], f32, tag="fp")
                for t in range(HPC):
                    nc.tensor.matmul(
                        fp_[:],
                        lhsT=wo_s[t][:, ob * 128 : (ob + 1) * 128],
                        rhs=ch[t][:, qc * 512 : (qc + 1) * 512],
                        start=(t == 0),
                        stop=(t == HPC - 1),
                    )
                fo = finp.tile([128, 512], f32, tag="fo")
                nc.vector.tensor_scalar_add(fo[:], fp_[:], boe_s[:, ob : ob + 1])
                nc.default_dma_engine.dma_start(
                    out=out[ob * 128 : (ob + 1) * 128, qc * 512 : (qc + 1) * 512],
                    in_=fo[:],
                )


_NC_CACHE = {}


def _get_nc(reps=1, use_cc=True):
    key = (reps, use_cc)
    if key not in _NC_CACHE:
        _NC_CACHE[key] = build(reps, use_cc)
    return _NC_CACHE[key]


def _prep_core_inputs(values, keys, query, wq, bq, wk, bk, wv, bv, wo, bo):
    """Build the 8 per-core input maps (host-side shard + layout prep)."""
    del bk  # cancels in softmax (per-column constant)
    m32 = (wq.T.astype(np.float64) @ wk.astype(np.float64)) / 32.0
    cvec = (wk.T.astype(np.float64) @ bq.astype(np.float64)) / 32.0
    wvt = wv.T.astype(np.float32).astype(ml_dtypes.bfloat16)

    bv_full = np.tile(bv, H)
    bo_eff = (
        bo.astype(np.float64) + wo.astype(np.float64) @ bv_full.astype(np.float64)
    ).astype(np.float32)
    woT = wo.T  # [in 1024, out 1024]

    # host-side Q projection: qp = q @ m32 + c  (folds wq, wk, bq, 1/32 scale)
    qall = query.reshape(B, S, H, D).astype(np.float32)
    qp_all = (
        qall.reshape(-1, D) @ m32.astype(np.float32)
    ).reshape(B, S, H, D) + cvec.astype(np.float32)

    in_maps = []
    ones = np.ones((HPC, NKB, 128, D), np.float32)
    for c in range(8):
        b, g = c // 2, c % 2
        heads = slice(g * HPC, (g + 1) * HPC)
        ktc = keys[b].reshape(S, H, D)[:, heads, :].transpose(1, 2, 0)
        qpc = qp_all[b][:, heads, :].transpose(1, 2, 0)
        vn = values[b].reshape(S, H, D)[:, heads, :].transpose(1, 0, 2)
        vn = vn.reshape(HPC, NKB, 128, D)
        vna = np.concatenate([vn, ones], axis=3)  # [h, kb, 128key, 128col]
        vna = vna.transpose(0, 2, 1, 3).reshape(HPC, 128, NKB * 128)
        ocols = slice(g * 512, (g + 1) * 512)
        wot = woT[:, ocols].reshape(HPC, 128, 512)[CHUNK_PERM]
        boe = np.ascontiguousarray(bo_eff[g * 512 : (g + 1) * 512].reshape(4, 128).T)
        in_maps.append(
            dict(
                kt=np.ascontiguousarray(ktc).astype(ml_dtypes.bfloat16),
                qp=np.ascontiguousarray(qpc).astype(ml_dtypes.bfloat16),
                vna=np.ascontiguousarray(vna).astype(ml_dtypes.bfloat16),
                wvt=wvt,
                wot=np.ascontiguousarray(wot).astype(ml_dtypes.bfloat16),
                boe=boe,
            )
        )
    return in_maps


def kernel(values, keys, query, wq, bq, wk, bk, wv, bv, wo, bo):
    values = np.asarray(values, np.float32)
    keys = np.asarray(keys, np.float32)
    query = np.asarray(query, np.float32)
    in_maps = _prep_core_inputs(
        values, keys, query,
        np.asarray(wq, np.float32), np.asarray(bq, np.float32),
        np.asarray(wk, np.float32), np.asarray(bk, np.float32),
        np.asarray(wv, np.float32), np.asarray(bv, np.float32),
        np.asarray(wo, np.float32), np.asarray(bo, np.float32),
    )
    nc = _get_nc()
    res = run_bass_kernel_spmd(nc, in_maps, list(range(8)))
    out = np.empty((B, S, 1024), np.float32)
    for c in range(8):
        b, g = c // 2, c % 2
        out[b, :, g * 512 : (g + 1) * 512] = res.results[c]["out"].T
    return out
